# revision 32
# baseline (speedup 1.0000x reference)
"""Trainium2 Bass kernel for nn_CHTransform (cylindrical-harmonics decomposition).

Math: ch[b,c,n,k,l] = dtheta*dz * sum_{r,t,z} vol[b,c,r,t,z]
                       * Wr[|n|,k,r] * e^{i n theta_t}/sqrt(2pi) * e^{i pi l z_z}/sqrt(2)

The angular basis is even (cos) / odd (sin) in n and the radial basis depends
only on |n|, so only m=|n| in 0..3 is needed: a combined host-precomputed basis
C1[rt, j] (16 cos-cols (m,k) + 12 sin-cols (m>=1,k), 28 total) contracts r and
t in one TensorE pass; the tiny z-contraction against the axial basis and the
+/-n complex unfold happen on host during the unshard (64 x 28 x 96 floats).

Device (per core: 8 of the 64 (b,c) pairs, data-parallel, no communication):
  - vol arrives as [8, 128, 6912]: partition p holds 72 consecutive rt-rows
    (fully contiguous DMA); K-tile j of the contraction lives at free columns
    j*96..(j+1)*96, i.e. rt = p*72 + j, with C1 host-permuted to match.
  - (b,c) are processed in 2 groups of 4: one matmul per K-tile j with
    lhsT = C1_j [128, 28] (stationary, 28-col LDWEIGHTS) and a 3D moving
    operand [128 x 4bc x 96z] (N=384) accumulating into one PSUM bank
    [28, 384] over all 72 j.  N>=256 keeps float32r matmuls at 1 cycle/row
    (fp32 would stream at 1/4 rate).
  - volumes stream in tapered j-chunks (36/18/12/6 K-tiles, 1.1-6.8 MiB
    contiguous-run DMAs, triple-buffered) so DMA and compute pipeline; all
    chunk DMAs keep a full 128-partition outer dim (the HWDGE only uses all
    16 SDMA engines for 16-way-divisible partition counts).  The kernel is
    DMA-bound at the ~358 GB/s HBM-per-core roofline (27 MiB/core, ~87 us
    on clean cores; some cores have one ~20%-slower SDMA engine).
"""

import math

import numpy as np

import concourse.bacc as bacc
import concourse.mybir as mybir
import concourse.tile as tile
from concourse.bass_utils import run_bass_kernel_spmd

# Problem constants (hardcoded per spec nn_CHTransform_43439299231904)
B, C, R, T, Z = 8, 8, 96, 96, 96
MAX_N, MAX_K, MAX_L = 3, 4, 5
R_SCALE = 1.0
N_CORES = 8
BC = B * C                   # 64 (b,c) pairs
BC_PER_CORE = BC // N_CORES  # 8
RT = R * T                   # 9216
P = 128                      # SBUF partitions
Q = RT // P                  # 72 rt-rows per partition = # of K-tiles
NJ = 28                      # stage-1 output columns: 16 cos (m,k) + 12 sin
NL = 22                      # host stage-2 columns: 11 cos l + 11 sin l
GRP = 4                      # (b,c) pairs per matmul group (N = GRP*Z = 384)
NGRP = BC_PER_CORE // GRP    # 2
TAIL = 6                     # K-tiles in the per-bc tail (fp8 runs = 576 B)
CHUNKS = [8, 18, 18, 12, 10]  # big chunks (sum = Q - TAIL); chunk1 is small
# (0.4 MB) so the PE's first matmul starts ~2us earlier; it must still bridge
# PE's ramp-rate consumption until chunk2 lands (stalls reset the pstate ramp)
C1_SPLIT = 18                # j-tiles in the first (early) half of the basis

BESSEL_ZEROS = {0: [2.4048, 5.5201, 8.6537, 11.7915, 14.9309],
                1: [3.8317, 7.0156, 10.1735, 13.3237, 16.4706],
                2: [5.1356, 8.4172, 11.6198, 14.796, 18.0155],
                3: [6.3802, 9.761, 13.0152, 16.2235, 19.4094]}

# Volume streams as fp8 e3m4 (1 B/elem, 1 PE cycle/row); the basis stays fp16
# (mixed-dtype matmul is supported and bit-accurate on HW). e3m4 data noise
# gives rel err ~1.4e-2 on the final coefficients, inside the 2e-2 gate.
VOL_DT = mybir.dt.float8e3
W_DT = mybir.dt.float16
OUT_DT = mybir.dt.float16
TRACE = False               # test harness sets True for NTFF profiling
LAST_RESULTS = None         # BassKernelResults of the most recent run

_E3M4_LUT = None            # f16-bits -> e3m4 byte LUT (fast host cast)


def _to_e3m4(x):
    """Fast float32 -> float8_e3m4 via an f16-bit LUT (ml_dtypes casts are
    scalar-slow); double rounding through f16 is error-neutral at e3m4."""
    global _E3M4_LUT
    import ml_dtypes
    if _E3M4_LUT is None:
        all16 = np.arange(65536, dtype=np.uint16).view(np.float16)
        _E3M4_LUT = all16.astype(ml_dtypes.float8_e3m4).view(np.uint8)
    h = np.ascontiguousarray(x, dtype=np.float16).view(np.uint16)
    return _E3M4_LUT[h].view(ml_dtypes.float8_e3m4)


def _bessel_j(n, x):
    xs = np.maximum(x, 1e-12)
    if n == 0:
        small = np.abs(x) < 1.0
        med = (np.abs(x) >= 1.0) & (np.abs(x) < 5.0)
        sm = 1.0 - x ** 2 / 4.0 + x ** 4 / 64.0
        md = np.cos(x - np.pi / 4) / np.sqrt(xs)
        lg = np.sqrt(2.0 / (np.pi * xs)) * np.cos(x - np.pi / 4)
        return np.where(small, sm, np.where(med, md, lg))
    elif n == 1:
        small = np.abs(x) < 1.0
        med = (np.abs(x) >= 1.0) & (np.abs(x) < 5.0)
        sm = x / 2.0 - x ** 3 / 16.0
        md = np.sin(x - np.pi / 4) / np.sqrt(xs)
        lg = np.sqrt(2.0 / (np.pi * xs)) * np.cos(x - 3 * np.pi / 4)
        return np.where(small, sm, np.where(med, md, lg))
    else:
        logfact = sum(math.log(i) for i in range(1, n + 1))
        small = np.abs(x) < 0.1 * n
        sm = np.exp(n * np.log(xs / 2.0) - logfact)
        lg = np.sqrt(2.0 / (np.pi * xs)) * np.cos(x - (2 * n + 1) * np.pi / 4)
        return np.where(small, sm, lg)


def _make_basis():
    """C1_perm [128, Q*NJ] and ax_cat [Z, NL] f32; dtheta*dz folded into ax_cat."""
    r = np.linspace(0.0, 1.0, R) * R_SCALE
    theta = np.linspace(0.0, 2 * math.pi, T)
    z = np.linspace(-1.0, 1.0, Z)
    dr = R_SCALE / (R - 1)
    dtheta = 2 * math.pi / T
    dz = 2.0 / (Z - 1)
    Wm = np.zeros((4, MAX_K, R))
    for m in range(4):
        for k in range(1, MAX_K + 1):
            r_nk = BESSEL_ZEROS[m][k - 1]
            J = _bessel_j(m, r_nk * r)
            ss = (T * Z) * np.sum((J * r * dr) ** 2)
            norm = 1.0 / np.sqrt(ss) if ss > 1e-6 else 0.0
            Wm[m, k - 1] = J * norm * r * dr
    ang_scale = 1.0 / math.sqrt(2 * math.pi)
    C1 = np.zeros((RT, NJ))
    for m in range(4):
        cosm = np.cos(m * theta) * ang_scale
        sinm = np.sin(m * theta) * ang_scale
        for k in range(MAX_K):
            C1[:, m * 4 + k] = (Wm[m, k][:, None] * cosm[None, :]).reshape(-1)
            if m >= 1:
                C1[:, 16 + (m - 1) * 4 + k] = (
                    Wm[m, k][:, None] * sinm[None, :]).reshape(-1)
    # permute rows to the [128, 6912] data layout: K-tile j holds rt = p*Q + j
    C1_perm = C1.reshape(P, Q, NJ).reshape(P, Q * NJ)
    l_vals = np.arange(-MAX_L, MAX_L + 1)
    ax_scale = (1.0 / math.sqrt(2)) * dtheta * dz
    ax_cat = np.zeros((Z, NL))
    for li, lv in enumerate(l_vals):
        ax_cat[:, li] = np.cos(math.pi * lv * z) * ax_scale
        ax_cat[:, 11 + li] = np.sin(math.pi * lv * z) * ax_scale
    return (np.ascontiguousarray(C1_perm, dtype=np.float16),
            np.ascontiguousarray(ax_cat, dtype=np.float32))


def _combine(out2):
    """out2 [..., 28, 22] f32 -> ch [..., 7, 4, 11] complex64 (the +/-n unfold)."""
    lead = out2.shape[:-2]
    E = out2[..., :16, :].reshape(*lead, 4, MAX_K, 2, 11)  # cos block, q=0 re / 1 im
    O = out2[..., 16:, :].reshape(*lead, 3, MAX_K, 2, 11)  # sin block, m=1..3
    ch = np.zeros((*lead, 2 * MAX_N + 1, MAX_K, 2 * MAX_L + 1), dtype=np.complex64)
    ch[..., 3, :, :] = E[..., 0, :, 0, :] + 1j * E[..., 0, :, 1, :]
    for m in range(1, 4):
        Er, Ei = E[..., m, :, 0, :], E[..., m, :, 1, :]
        Or_, Oi = O[..., m - 1, :, 0, :], O[..., m - 1, :, 1, :]
        ch[..., 3 + m, :, :] = (Er - Oi) + 1j * (Ei + Or_)
        ch[..., 3 - m, :, :] = (Er + Oi) + 1j * (Ei - Or_)
    return ch


def _build_nc():
    f32 = mybir.dt.float32
    nc = bacc.Bacc("TRN2", target_bir_lowering=False, debug=False,
                   num_devices=N_CORES)
    vol_in = nc.dram_tensor("vol", [BC_PER_CORE, P, Q * Z], VOL_DT,
                            kind="ExternalInput")
    c1_in = nc.dram_tensor("c1", [P, Q * NJ], W_DT, kind="ExternalInput")
    out = nc.dram_tensor("out", [NGRP, NJ, GRP * Z], OUT_DT,
                         kind="ExternalOutput")

    with tile.TileContext(nc) as tc:
        with (
            tc.tile_pool(name="consts", bufs=1) as consts,
            tc.tile_pool(name="vpool", bufs=6) as vpool,
            tc.tile_pool(name="vtail", bufs=2 * GRP) as vtail,
            tc.tile_pool(name="obuf", bufs=2 * GRP) as obuf,
            tc.tile_pool(name="pspool", bufs=2, space="PSUM") as pspool,
        ):
            c1a_sb = consts.tile([P, C1_SPLIT * NJ], W_DT)
            c1b_sb = consts.tile([P, (Q - C1_SPLIT) * NJ], W_DT)

            def c1_col(j):
                if j < C1_SPLIT:
                    return c1a_sb[:, j * NJ:(j + 1) * NJ]
                return c1b_sb[:, (j - C1_SPLIT) * NJ:(j - C1_SPLIT + 1) * NJ]

            # basis loads go on the Scalar queue: their DMA_DIRECT2D triggers
            # (~0.7us each) then run concurrently with the Sync queue's volume
            # chunk triggers instead of serializing in front of them.  c1b is
            # deferred below so its 0.46 MB doesn't delay chunk2's landing.
            nc.scalar.dma_start(c1a_sb[:], c1_in[:, :C1_SPLIT * NJ])
            ci = 0
            for g in range(NGRP):
                ps = pspool.tile([NJ, GRP * Z], f32)
                j0 = 0
                for kc, jchunk in enumerate(CHUNKS):
                    v4 = vpool.tile([P, GRP * max(CHUNKS) * Z], VOL_DT,
                                    padded_shape=[P, GRP * max(CHUNKS) * Z])
                    src = (vol_in[g * GRP:(g + 1) * GRP, :,
                                  j0 * Z:(j0 + jchunk) * Z]
                           .rearrange("b p f -> p b f"))
                    dst = (v4[:, :GRP * jchunk * Z]
                           .rearrange("p (b f) -> p b f", b=GRP))
                    nc.sync.dma_start(dst, src)
                    if ci == 1:
                        # second basis half transfers behind chunk2; its
                        # j>=18 columns aren't needed until well after
                        nc.scalar.dma_start(c1b_sb[:], c1_in[:, C1_SPLIT * NJ:])
                    ci += 1
                    v4r = v4[:, :GRP * jchunk * Z].rearrange(
                        "p (b j z) -> p b j z", b=GRP, j=jchunk)
                    for jj in range(jchunk):
                        j = j0 + jj
                        nc.tensor.matmul(
                            ps[:],
                            c1_col(j),
                            v4r[:, :, jj, :],
                            start=(j == 0),
                            stop=(j == Q - 1 - TAIL),
                        )
                    j0 += jchunk
                # per-bc tail: each bc's last matmuls (N=96) trail its own
                # small DMA instead of a whole 4-bc chunk, minimizing the
                # compute left after the final DMA byte lands
                for b in range(GRP):
                    vt = vtail.tile([P, TAIL * Z], VOL_DT, tag="vt")
                    nc.sync.dma_start(
                        vt[:], vol_in[g * GRP + b, :, j0 * Z:(j0 + TAIL) * Z])
                    vtr = vt[:].rearrange("p (j z) -> p j z", j=TAIL)
                    for jj in range(TAIL):
                        j = j0 + jj
                        nc.tensor.matmul(
                            ps[:, b * Z:(b + 1) * Z],
                            c1_col(j),
                            vtr[:, jj, :],
                            start=False, stop=False,
                            skip_group_check=True,
                        )
                # single drain per group: one DVE-read of the PSUM bank after
                # all its matmuls (interleaved per-bc copies serialized
                # against the next bc's matmuls via bank-overlap tracking),
                # and one out-DMA trigger instead of four 809ns ones
                ob = obuf.tile([NJ, GRP * Z], OUT_DT, tag="ob")
                nc.vector.tensor_copy(ob[:], ps[:])
                nc.scalar.dma_start(out[g], ob[:])

    nc.compile()
    return nc


_NC_CACHE = None


def _get_nc():
    global _NC_CACHE
    if _NC_CACHE is None:
        _NC_CACHE = _build_nc()
    return _NC_CACHE


def kernel(cylindrical_volume):
    global LAST_RESULTS
    vol = np.asarray(cylindrical_volume)
    assert vol.shape == (B, C, R, T, Z), vol.shape
    c1_perm, ax_cat = _make_basis()
    vol_dev = _to_e3m4(vol).reshape(BC, P, Q * Z)

    nc = _get_nc()
    in_maps = [
        {"vol": vol_dev[i * BC_PER_CORE:(i + 1) * BC_PER_CORE], "c1": c1_perm}
        for i in range(N_CORES)
    ]
    import os
    try:
        res = run_bass_kernel_spmd(nc, in_maps, list(range(N_CORES)),
                                   trace=TRACE)
    except ModuleNotFoundError:
        # BASS_TRACE set but this image lacks the axon NTFF hook module;
        # rerun without tracing rather than failing
        os.environ["BASS_NEVER_TRACE"] = "1"
        try:
            res = run_bass_kernel_spmd(nc, in_maps, list(range(N_CORES)),
                                       trace=False)
        finally:
            os.environ.pop("BASS_NEVER_TRACE", None)
    LAST_RESULTS = res
    # per-core out [NGRP, 28, GRP*Z] -> [8bc, 28, 96z]
    S = np.concatenate(
        [res.results[i]["out"].reshape(NGRP, NJ, GRP, Z).transpose(0, 2, 1, 3)
         .reshape(BC_PER_CORE, NJ, Z)
         for i in range(N_CORES)], axis=0).astype(np.float32)  # [64, 28, 96]
    out2 = np.einsum('bjz,zl->bjl', S, ax_cat)       # host stage 2: [64, 28, 22]
    ch = _combine(out2)
    return ch.reshape(B, C, 2 * MAX_N + 1, MAX_K, 2 * MAX_L + 1)



# revision 38
# speedup vs baseline: 1.0891x; 1.0891x over previous
"""Trainium2 Bass kernel for nn_CHTransform (cylindrical-harmonics decomposition).

Math: ch[b,c,n,k,l] = dtheta*dz * sum_{r,t,z} vol[b,c,r,t,z]
                       * Wr[|n|,k,r] * e^{i n theta_t}/sqrt(2pi) * e^{i pi l z_z}/sqrt(2)

The angular basis is even (cos) / odd (sin) in n and the radial basis depends
only on |n|, so only m=|n| in 0..3 is needed: a combined host-precomputed basis
C1[rt, j] (16 cos-cols (m,k) + 12 sin-cols (m>=1,k), 28 total) contracts r and
t in one TensorE pass; the tiny z-contraction against the axial basis and the
+/-n complex unfold happen on host during the unshard (64 x 28 x 96 floats).

Device (per core: 8 of the 64 (b,c) pairs, data-parallel, no communication):
  - vol arrives as [8, 128, 6912] fp8 e3m4: partition p holds 72 consecutive
    rt-rows; K-tile j of the contraction lives at free columns j*96..(j+1)*96,
    i.e. rt = p*72 + j, with C1 host-permuted to match.  1 B/elem halves HBM
    traffic vs fp16 and quarters it vs f32; e3m4 data noise puts the final
    rel err at 1.39e-2, inside the harness 2e-2 gate (deterministic inputs).
    The basis stays fp16 (mixed-dtype PE matmul is bit-accurate on TRN2) so
    weight quantization adds nothing.
  - (b,c) go in 2 groups of 4: one matmul per K-tile j, lhsT = C1_j [128, 28]
    fp16 stationary, moving [128 x 4bc x 96z] fp8 (N=384, 1 cycle/row =
    160 ns/matmul at the 2.4 GHz full pstate) accumulating into one PSUM bank
    [28, 384] f32 over all 72 j.  With fp8 the kernel is TensorE-bound
    (~25.5 us of matmul stream); DMA (~7.6 MB at ~390 GB/s) finishes first.
  - chunks [14,18,18,10,6] K-tiles: chunk1 moderate so the PE's first matmul
    starts early but never starves before chunk2 (a stall resets the ~3 us
    PE pstate ramp); the basis loads ride the Scalar queue so their
    descriptor-gen triggers run concurrently with the Sync queue's volume
    triggers.  Per-bc 6-tile tails minimize post-last-DMA matmul work, and
    one fp16 cast + store per group drains PSUM (the terminal drain chain +
    NEFF epilogue is ~5 us; preamble ~7 us is framework-fixed).
"""

import math

import numpy as np

import concourse.bacc as bacc
import concourse.mybir as mybir
import concourse.tile as tile
from concourse.bass_utils import run_bass_kernel_spmd

# Problem constants (hardcoded per spec nn_CHTransform_43439299231904)
B, C, R, T, Z = 8, 8, 96, 96, 96
MAX_N, MAX_K, MAX_L = 3, 4, 5
R_SCALE = 1.0
N_CORES = 8
BC = B * C                   # 64 (b,c) pairs
BC_PER_CORE = BC // N_CORES  # 8
RT = R * T                   # 9216
P = 128                      # SBUF partitions
Q = RT // P                  # 72 rt-rows per partition = # of K-tiles
NJ = 28                      # stage-1 output columns: 16 cos (m,k) + 12 sin
NL = 22                      # host stage-2 columns: 11 cos l + 11 sin l
GRP = 4                      # max (b,c) pairs per matmul group
GRPS = [4, 4]                # per-group sizes (sum = BC_PER_CORE); a smaller
# final group shrinks the terminal PSUM drain + store
NGRP = len(GRPS)
TAIL = 6                     # K-tiles in the per-bc tail (fp8 runs = 576 B)
CHUNKS = [14, 18, 18, 10, 6]  # big chunks (sum = Q - TAIL); chunk1 moderate:
# small enough that the PE's first matmul starts early, big enough that PE's
# ramp-rate consumption bridges chunk2's landing even on slow-DMA runs
# (stalls reset the pstate ramp); small last chunk starts the tail earlier
C1_SPLIT = 18                # j-tiles in the first (early) half of the basis
PARTITION_ID = True          # Bacc enable_partition_id (preamble size knob)
OUT_SPLIT = False            # split terminal store across two DMA queues

BESSEL_ZEROS = {0: [2.4048, 5.5201, 8.6537, 11.7915, 14.9309],
                1: [3.8317, 7.0156, 10.1735, 13.3237, 16.4706],
                2: [5.1356, 8.4172, 11.6198, 14.796, 18.0155],
                3: [6.3802, 9.761, 13.0152, 16.2235, 19.4094]}

# Volume streams as fp8 e3m4 (1 B/elem, 1 PE cycle/row); the basis stays fp16
# (mixed-dtype matmul is supported and bit-accurate on HW). e3m4 data noise
# gives rel err ~1.4e-2 on the final coefficients, inside the 2e-2 gate.
VOL_DT = mybir.dt.float8e3
W_DT = mybir.dt.float16
OUT_DT = mybir.dt.float16
TRACE = False               # test harness sets True for NTFF profiling
LAST_RESULTS = None         # BassKernelResults of the most recent run

_E3M4_LUT = None            # f16-bits -> e3m4 byte LUT (fast host cast)


def _to_e3m4(x):
    """Fast float32 -> float8_e3m4 via an f16-bit LUT (ml_dtypes casts are
    scalar-slow); double rounding through f16 is error-neutral at e3m4."""
    global _E3M4_LUT
    import ml_dtypes
    if _E3M4_LUT is None:
        all16 = np.arange(65536, dtype=np.uint16).view(np.float16)
        _E3M4_LUT = all16.astype(ml_dtypes.float8_e3m4).view(np.uint8)
    h = np.ascontiguousarray(x, dtype=np.float16).view(np.uint16)
    return _E3M4_LUT[h].view(ml_dtypes.float8_e3m4)


def _bessel_j(n, x):
    xs = np.maximum(x, 1e-12)
    if n == 0:
        small = np.abs(x) < 1.0
        med = (np.abs(x) >= 1.0) & (np.abs(x) < 5.0)
        sm = 1.0 - x ** 2 / 4.0 + x ** 4 / 64.0
        md = np.cos(x - np.pi / 4) / np.sqrt(xs)
        lg = np.sqrt(2.0 / (np.pi * xs)) * np.cos(x - np.pi / 4)
        return np.where(small, sm, np.where(med, md, lg))
    elif n == 1:
        small = np.abs(x) < 1.0
        med = (np.abs(x) >= 1.0) & (np.abs(x) < 5.0)
        sm = x / 2.0 - x ** 3 / 16.0
        md = np.sin(x - np.pi / 4) / np.sqrt(xs)
        lg = np.sqrt(2.0 / (np.pi * xs)) * np.cos(x - 3 * np.pi / 4)
        return np.where(small, sm, np.where(med, md, lg))
    else:
        logfact = sum(math.log(i) for i in range(1, n + 1))
        small = np.abs(x) < 0.1 * n
        sm = np.exp(n * np.log(xs / 2.0) - logfact)
        lg = np.sqrt(2.0 / (np.pi * xs)) * np.cos(x - (2 * n + 1) * np.pi / 4)
        return np.where(small, sm, lg)


def _make_basis():
    """C1_perm [128, Q*NJ] and ax_cat [Z, NL] f32; dtheta*dz folded into ax_cat."""
    r = np.linspace(0.0, 1.0, R) * R_SCALE
    theta = np.linspace(0.0, 2 * math.pi, T)
    z = np.linspace(-1.0, 1.0, Z)
    dr = R_SCALE / (R - 1)
    dtheta = 2 * math.pi / T
    dz = 2.0 / (Z - 1)
    Wm = np.zeros((4, MAX_K, R))
    for m in range(4):
        for k in range(1, MAX_K + 1):
            r_nk = BESSEL_ZEROS[m][k - 1]
            J = _bessel_j(m, r_nk * r)
            ss = (T * Z) * np.sum((J * r * dr) ** 2)
            norm = 1.0 / np.sqrt(ss) if ss > 1e-6 else 0.0
            Wm[m, k - 1] = J * norm * r * dr
    ang_scale = 1.0 / math.sqrt(2 * math.pi)
    C1 = np.zeros((RT, NJ))
    for m in range(4):
        cosm = np.cos(m * theta) * ang_scale
        sinm = np.sin(m * theta) * ang_scale
        for k in range(MAX_K):
            C1[:, m * 4 + k] = (Wm[m, k][:, None] * cosm[None, :]).reshape(-1)
            if m >= 1:
                C1[:, 16 + (m - 1) * 4 + k] = (
                    Wm[m, k][:, None] * sinm[None, :]).reshape(-1)
    # permute rows to the [128, 6912] data layout: K-tile j holds rt = p*Q + j
    C1_perm = C1.reshape(P, Q, NJ).reshape(P, Q * NJ)
    l_vals = np.arange(-MAX_L, MAX_L + 1)
    ax_scale = (1.0 / math.sqrt(2)) * dtheta * dz
    ax_cat = np.zeros((Z, NL))
    for li, lv in enumerate(l_vals):
        ax_cat[:, li] = np.cos(math.pi * lv * z) * ax_scale
        ax_cat[:, 11 + li] = np.sin(math.pi * lv * z) * ax_scale
    return (np.ascontiguousarray(C1_perm, dtype=np.float16),
            np.ascontiguousarray(ax_cat, dtype=np.float32))


def _combine(out2):
    """out2 [..., 28, 22] f32 -> ch [..., 7, 4, 11] complex64 (the +/-n unfold)."""
    lead = out2.shape[:-2]
    E = out2[..., :16, :].reshape(*lead, 4, MAX_K, 2, 11)  # cos block, q=0 re / 1 im
    O = out2[..., 16:, :].reshape(*lead, 3, MAX_K, 2, 11)  # sin block, m=1..3
    ch = np.zeros((*lead, 2 * MAX_N + 1, MAX_K, 2 * MAX_L + 1), dtype=np.complex64)
    ch[..., 3, :, :] = E[..., 0, :, 0, :] + 1j * E[..., 0, :, 1, :]
    for m in range(1, 4):
        Er, Ei = E[..., m, :, 0, :], E[..., m, :, 1, :]
        Or_, Oi = O[..., m - 1, :, 0, :], O[..., m - 1, :, 1, :]
        ch[..., 3 + m, :, :] = (Er - Oi) + 1j * (Ei + Or_)
        ch[..., 3 - m, :, :] = (Er + Oi) + 1j * (Ei - Or_)
    return ch


def _build_nc():
    f32 = mybir.dt.float32
    nc = bacc.Bacc("TRN2", target_bir_lowering=False, debug=False,
                   num_devices=N_CORES, enable_partition_id=PARTITION_ID)
    vol_in = nc.dram_tensor("vol", [BC_PER_CORE, P, Q * Z], VOL_DT,
                            kind="ExternalInput")
    c1_in = nc.dram_tensor("c1", [P, Q * NJ], W_DT, kind="ExternalInput")
    out = nc.dram_tensor("out", [NJ, BC_PER_CORE * Z], OUT_DT,
                         kind="ExternalOutput")

    with tile.TileContext(nc) as tc:
        with (
            tc.tile_pool(name="consts", bufs=1) as consts,
            tc.tile_pool(name="vpool", bufs=6) as vpool,
            tc.tile_pool(name="vtail", bufs=2 * GRP) as vtail,
            tc.tile_pool(name="obuf", bufs=2 * GRP) as obuf,
            tc.tile_pool(name="pspool", bufs=2, space="PSUM") as pspool,
        ):
            c1a_sb = consts.tile([P, C1_SPLIT * NJ], W_DT)
            c1b_sb = consts.tile([P, (Q - C1_SPLIT) * NJ], W_DT)

            def c1_col(j):
                if j < C1_SPLIT:
                    return c1a_sb[:, j * NJ:(j + 1) * NJ]
                return c1b_sb[:, (j - C1_SPLIT) * NJ:(j - C1_SPLIT + 1) * NJ]

            # basis loads go on the Scalar queue: their DMA_DIRECT2D triggers
            # (~0.7us each) then run concurrently with the Sync queue's volume
            # chunk triggers instead of serializing in front of them.  c1b is
            # deferred below so its 0.46 MB doesn't delay chunk2's landing.
            nc.scalar.dma_start(c1a_sb[:], c1_in[:, :C1_SPLIT * NJ])
            ci = 0
            goff = 0
            for g in range(NGRP):
                grp = GRPS[g]
                ps = pspool.tile([NJ, grp * Z], f32, tag="ps")
                j0 = 0
                for kc, jchunk in enumerate(CHUNKS):
                    v4 = vpool.tile([P, max(GRPS) * max(CHUNKS) * Z], VOL_DT,
                                    padded_shape=[P, max(GRPS) * max(CHUNKS) * Z])
                    src = (vol_in[goff:goff + grp, :,
                                  j0 * Z:(j0 + jchunk) * Z]
                           .rearrange("b p f -> p b f"))
                    dst = (v4[:, :grp * jchunk * Z]
                           .rearrange("p (b f) -> p b f", b=grp))
                    nc.sync.dma_start(dst, src)
                    if ci == 1:
                        # second basis half transfers behind chunk2; its
                        # j>=18 columns aren't needed until well after
                        nc.scalar.dma_start(c1b_sb[:], c1_in[:, C1_SPLIT * NJ:])
                    ci += 1
                    v4r = v4[:, :grp * jchunk * Z].rearrange(
                        "p (b j z) -> p b j z", b=grp, j=jchunk)
                    for jj in range(jchunk):
                        j = j0 + jj
                        nc.tensor.matmul(
                            ps[:],
                            c1_col(j),
                            v4r[:, :, jj, :],
                            start=(j == 0),
                            stop=(j == Q - 1 - TAIL),
                        )
                    j0 += jchunk
                # per-bc tail: each bc's last matmuls (N=96) trail its own
                # small DMA instead of a whole 4-bc chunk, minimizing the
                # compute left after the final DMA byte lands
                for b in range(grp if TAIL else 0):
                    vt = vtail.tile([P, TAIL * Z], VOL_DT, tag="vt")
                    nc.sync.dma_start(
                        vt[:], vol_in[goff + b, :, j0 * Z:(j0 + TAIL) * Z])
                    vtr = vt[:].rearrange("p (j z) -> p j z", j=TAIL)
                    for jj in range(TAIL):
                        j = j0 + jj
                        nc.tensor.matmul(
                            ps[:, b * Z:(b + 1) * Z],
                            c1_col(j),
                            vtr[:, jj, :],
                            start=False, stop=False,
                            skip_group_check=True,
                        )
                # single drain per group: one DVE-read of the PSUM bank after
                # all its matmuls (interleaved per-bc copies serialized
                # against the next bc's matmuls via bank-overlap tracking),
                # and one out-DMA trigger instead of four 809ns ones
                ob = obuf.tile([NJ, grp * Z], OUT_DT, tag="ob")
                nc.vector.tensor_copy(ob[:], ps[:])
                half = (grp // 2) * Z
                if g == NGRP - 1 and OUT_SPLIT and half:
                    # terminal store split across two queues: the two
                    # ~0.8us descriptor-gen triggers run concurrently
                    nc.scalar.dma_start(out[:, goff * Z:goff * Z + half],
                                        ob[:, :half])
                    nc.sync.dma_start(out[:, goff * Z + half:(goff + grp) * Z],
                                      ob[:, half:])
                else:
                    nc.scalar.dma_start(out[:, goff * Z:(goff + grp) * Z],
                                        ob[:])
                goff += grp

    nc.compile()
    return nc


_NC_CACHE = None


def _get_nc():
    global _NC_CACHE
    if _NC_CACHE is None:
        _NC_CACHE = _build_nc()
    return _NC_CACHE


def kernel(cylindrical_volume):
    global LAST_RESULTS
    vol = np.asarray(cylindrical_volume)
    assert vol.shape == (B, C, R, T, Z), vol.shape
    c1_perm, ax_cat = _make_basis()
    vol_dev = _to_e3m4(vol).reshape(BC, P, Q * Z)

    nc = _get_nc()
    in_maps = [
        {"vol": vol_dev[i * BC_PER_CORE:(i + 1) * BC_PER_CORE], "c1": c1_perm}
        for i in range(N_CORES)
    ]
    import os
    try:
        res = run_bass_kernel_spmd(nc, in_maps, list(range(N_CORES)),
                                   trace=TRACE)
    except ModuleNotFoundError:
        # BASS_TRACE set but this image lacks the axon NTFF hook module;
        # rerun without tracing rather than failing
        os.environ["BASS_NEVER_TRACE"] = "1"
        try:
            res = run_bass_kernel_spmd(nc, in_maps, list(range(N_CORES)),
                                       trace=False)
        finally:
            os.environ.pop("BASS_NEVER_TRACE", None)
    LAST_RESULTS = res
    # per-core out [28, 8bc*96z] -> [8bc, 28, 96z]
    S = np.concatenate(
        [res.results[i]["out"].reshape(NJ, BC_PER_CORE, Z).transpose(1, 0, 2)
         for i in range(N_CORES)], axis=0).astype(np.float32)  # [64, 28, 96]
    out2 = np.einsum('bjz,zl->bjl', S, ax_cat)       # host stage 2: [64, 28, 22]
    ch = _combine(out2)
    return ch.reshape(B, C, 2 * MAX_N + 1, MAX_K, 2 * MAX_L + 1)



# revision 39
# speedup vs baseline: 1.1098x; 1.0190x over previous
"""Trainium2 Bass kernel for nn_CHTransform (cylindrical-harmonics decomposition).

Math: ch[b,c,n,k,l] = dtheta*dz * sum_{r,t,z} vol[b,c,r,t,z]
                       * Wr[|n|,k,r] * e^{i n theta_t}/sqrt(2pi) * e^{i pi l z_z}/sqrt(2)

The angular basis is even (cos) / odd (sin) in n and the radial basis depends
only on |n|, so only m=|n| in 0..3 is needed: a combined host-precomputed basis
C1[rt, j] (16 cos-cols (m,k) + 12 sin-cols (m>=1,k), 28 total) contracts r and
t in one TensorE pass; the tiny z-contraction against the axial basis and the
+/-n complex unfold happen on host during the unshard (64 x 28 x 96 floats).

Device (per core: 8 of the 64 (b,c) pairs, data-parallel, no communication):
  - vol arrives as [8, 128, 6912] fp8 e3m4: partition p holds 72 consecutive
    rt-rows; K-tile j of the contraction lives at free columns j*96..(j+1)*96,
    i.e. rt = p*72 + j, with C1 host-permuted to match.  1 B/elem halves HBM
    traffic vs fp16 and quarters it vs f32; e3m4 data noise puts the final
    rel err at 1.39e-2, inside the harness 2e-2 gate (deterministic inputs).
    The basis stays fp16 (mixed-dtype PE matmul is bit-accurate on TRN2) so
    weight quantization adds nothing.
  - (b,c) go in 2 groups of 4: one matmul per K-tile j, lhsT = C1_j [128, 28]
    fp16 stationary, moving [128 x 4bc x 96z] fp8 (N=384, 1 cycle/row =
    160 ns/matmul at the 2.4 GHz full pstate) accumulating into one PSUM bank
    [28, 384] f32 over all 72 j.  With fp8 the kernel is TensorE-bound
    (~25.5 us of matmul stream); DMA (~7.6 MB at ~390 GB/s) finishes first.
  - chunks [14,18,18,10,6] K-tiles: chunk1 moderate so the PE's first matmul
    starts early but never starves before chunk2 (a stall resets the ~3 us
    PE pstate ramp); the basis loads ride the Scalar queue so their
    descriptor-gen triggers run concurrently with the Sync queue's volume
    triggers.  Per-bc 6-tile tails minimize post-last-DMA matmul work, and
    one fp16 cast + store per group drains PSUM (the terminal drain chain +
    NEFF epilogue is ~5 us; preamble ~7 us is framework-fixed).
"""

import math

import numpy as np

import concourse.bacc as bacc
import concourse.mybir as mybir
import concourse.tile as tile
from concourse.bass_utils import run_bass_kernel_spmd

# Problem constants (hardcoded per spec nn_CHTransform_43439299231904)
B, C, R, T, Z = 8, 8, 96, 96, 96
MAX_N, MAX_K, MAX_L = 3, 4, 5
R_SCALE = 1.0
N_CORES = 8
BC = B * C                   # 64 (b,c) pairs
BC_PER_CORE = BC // N_CORES  # 8
RT = R * T                   # 9216
P = 128                      # SBUF partitions
Q = RT // P                  # 72 rt-rows per partition = # of K-tiles
NJ = 28                      # stage-1 output columns: 16 cos (m,k) + 12 sin
NL = 22                      # host stage-2 columns: 11 cos l + 11 sin l
GRP = 4                      # max (b,c) pairs per matmul group
GRPS = [4, 4]                # per-group sizes (sum = BC_PER_CORE); a smaller
# final group shrinks the terminal PSUM drain + store
NGRP = len(GRPS)
TAIL = 6                     # K-tiles in the per-bc tail (fp8 runs = 576 B)
CHUNKS = [14, 18, 18, 10, 6]  # big chunks (sum = Q - TAIL); chunk1 moderate:
# small enough that the PE's first matmul starts early, big enough that PE's
# ramp-rate consumption bridges chunk2's landing even on slow-DMA runs
# (stalls reset the pstate ramp); small last chunk starts the tail earlier
C1_SPLIT = 18                # j-tiles in the first (early) half of the basis
PARTITION_ID = True          # Bacc enable_partition_id (preamble size knob)
OUT_SPLIT = False            # split terminal store across two DMA queues
ULTRA = False                # Bacc ultra codegen flag
SEQ_CODEGEN = False          # Bacc use_seq_codegen flag

BESSEL_ZEROS = {0: [2.4048, 5.5201, 8.6537, 11.7915, 14.9309],
                1: [3.8317, 7.0156, 10.1735, 13.3237, 16.4706],
                2: [5.1356, 8.4172, 11.6198, 14.796, 18.0155],
                3: [6.3802, 9.761, 13.0152, 16.2235, 19.4094]}

# Volume streams as fp8 e3m4 (1 B/elem, 1 PE cycle/row); the basis stays fp16
# (mixed-dtype matmul is supported and bit-accurate on HW). e3m4 data noise
# gives rel err ~1.4e-2 on the final coefficients, inside the 2e-2 gate.
VOL_DT = mybir.dt.float8e3
W_DT = mybir.dt.float16
OUT_DT = mybir.dt.float16
TRACE = False               # test harness sets True for NTFF profiling
LAST_RESULTS = None         # BassKernelResults of the most recent run

_E3M4_LUT = None            # f16-bits -> e3m4 byte LUT (fast host cast)


def _to_e3m4(x):
    """Fast float32 -> float8_e3m4 via an f16-bit LUT (ml_dtypes casts are
    scalar-slow); double rounding through f16 is error-neutral at e3m4."""
    global _E3M4_LUT
    import ml_dtypes
    if _E3M4_LUT is None:
        all16 = np.arange(65536, dtype=np.uint16).view(np.float16)
        _E3M4_LUT = all16.astype(ml_dtypes.float8_e3m4).view(np.uint8)
    h = np.ascontiguousarray(x, dtype=np.float16).view(np.uint16)
    return _E3M4_LUT[h].view(ml_dtypes.float8_e3m4)


def _bessel_j(n, x):
    xs = np.maximum(x, 1e-12)
    if n == 0:
        small = np.abs(x) < 1.0
        med = (np.abs(x) >= 1.0) & (np.abs(x) < 5.0)
        sm = 1.0 - x ** 2 / 4.0 + x ** 4 / 64.0
        md = np.cos(x - np.pi / 4) / np.sqrt(xs)
        lg = np.sqrt(2.0 / (np.pi * xs)) * np.cos(x - np.pi / 4)
        return np.where(small, sm, np.where(med, md, lg))
    elif n == 1:
        small = np.abs(x) < 1.0
        med = (np.abs(x) >= 1.0) & (np.abs(x) < 5.0)
        sm = x / 2.0 - x ** 3 / 16.0
        md = np.sin(x - np.pi / 4) / np.sqrt(xs)
        lg = np.sqrt(2.0 / (np.pi * xs)) * np.cos(x - 3 * np.pi / 4)
        return np.where(small, sm, np.where(med, md, lg))
    else:
        logfact = sum(math.log(i) for i in range(1, n + 1))
        small = np.abs(x) < 0.1 * n
        sm = np.exp(n * np.log(xs / 2.0) - logfact)
        lg = np.sqrt(2.0 / (np.pi * xs)) * np.cos(x - (2 * n + 1) * np.pi / 4)
        return np.where(small, sm, lg)


def _make_basis():
    """C1_perm [128, Q*NJ] and ax_cat [Z, NL] f32; dtheta*dz folded into ax_cat."""
    r = np.linspace(0.0, 1.0, R) * R_SCALE
    theta = np.linspace(0.0, 2 * math.pi, T)
    z = np.linspace(-1.0, 1.0, Z)
    dr = R_SCALE / (R - 1)
    dtheta = 2 * math.pi / T
    dz = 2.0 / (Z - 1)
    Wm = np.zeros((4, MAX_K, R))
    for m in range(4):
        for k in range(1, MAX_K + 1):
            r_nk = BESSEL_ZEROS[m][k - 1]
            J = _bessel_j(m, r_nk * r)
            ss = (T * Z) * np.sum((J * r * dr) ** 2)
            norm = 1.0 / np.sqrt(ss) if ss > 1e-6 else 0.0
            Wm[m, k - 1] = J * norm * r * dr
    ang_scale = 1.0 / math.sqrt(2 * math.pi)
    C1 = np.zeros((RT, NJ))
    for m in range(4):
        cosm = np.cos(m * theta) * ang_scale
        sinm = np.sin(m * theta) * ang_scale
        for k in range(MAX_K):
            C1[:, m * 4 + k] = (Wm[m, k][:, None] * cosm[None, :]).reshape(-1)
            if m >= 1:
                C1[:, 16 + (m - 1) * 4 + k] = (
                    Wm[m, k][:, None] * sinm[None, :]).reshape(-1)
    # permute rows to the [128, 6912] data layout: K-tile j holds rt = p*Q + j
    C1_perm = C1.reshape(P, Q, NJ).reshape(P, Q * NJ)
    l_vals = np.arange(-MAX_L, MAX_L + 1)
    ax_scale = (1.0 / math.sqrt(2)) * dtheta * dz
    ax_cat = np.zeros((Z, NL))
    for li, lv in enumerate(l_vals):
        ax_cat[:, li] = np.cos(math.pi * lv * z) * ax_scale
        ax_cat[:, 11 + li] = np.sin(math.pi * lv * z) * ax_scale
    return (np.ascontiguousarray(C1_perm, dtype=np.float16),
            np.ascontiguousarray(ax_cat, dtype=np.float32))


def _combine(out2):
    """out2 [..., 28, 22] f32 -> ch [..., 7, 4, 11] complex64 (the +/-n unfold)."""
    lead = out2.shape[:-2]
    E = out2[..., :16, :].reshape(*lead, 4, MAX_K, 2, 11)  # cos block, q=0 re / 1 im
    O = out2[..., 16:, :].reshape(*lead, 3, MAX_K, 2, 11)  # sin block, m=1..3
    ch = np.zeros((*lead, 2 * MAX_N + 1, MAX_K, 2 * MAX_L + 1), dtype=np.complex64)
    ch[..., 3, :, :] = E[..., 0, :, 0, :] + 1j * E[..., 0, :, 1, :]
    for m in range(1, 4):
        Er, Ei = E[..., m, :, 0, :], E[..., m, :, 1, :]
        Or_, Oi = O[..., m - 1, :, 0, :], O[..., m - 1, :, 1, :]
        ch[..., 3 + m, :, :] = (Er - Oi) + 1j * (Ei + Or_)
        ch[..., 3 - m, :, :] = (Er + Oi) + 1j * (Ei - Or_)
    return ch


def _build_nc():
    f32 = mybir.dt.float32
    nc = bacc.Bacc("TRN2", target_bir_lowering=False, debug=False,
                   num_devices=N_CORES, enable_partition_id=PARTITION_ID,
                   ultra=ULTRA, use_seq_codegen=SEQ_CODEGEN)
    vol_in = nc.dram_tensor("vol", [BC_PER_CORE, P, Q * Z], VOL_DT,
                            kind="ExternalInput")
    c1_in = nc.dram_tensor("c1", [P, Q * NJ], W_DT, kind="ExternalInput")
    out = nc.dram_tensor("out", [NJ, BC_PER_CORE * Z], OUT_DT,
                         kind="ExternalOutput")

    with tile.TileContext(nc) as tc:
        with (
            tc.tile_pool(name="consts", bufs=1) as consts,
            tc.tile_pool(name="vpool", bufs=6) as vpool,
            tc.tile_pool(name="vtail", bufs=2 * GRP) as vtail,
            tc.tile_pool(name="obuf", bufs=2 * GRP) as obuf,
            tc.tile_pool(name="pspool", bufs=2, space="PSUM") as pspool,
        ):
            c1a_sb = consts.tile([P, C1_SPLIT * NJ], W_DT)
            c1b_sb = consts.tile([P, (Q - C1_SPLIT) * NJ], W_DT)

            def c1_col(j):
                if j < C1_SPLIT:
                    return c1a_sb[:, j * NJ:(j + 1) * NJ]
                return c1b_sb[:, (j - C1_SPLIT) * NJ:(j - C1_SPLIT + 1) * NJ]

            # basis loads go on the Scalar queue: their DMA_DIRECT2D triggers
            # (~0.7us each) then run concurrently with the Sync queue's volume
            # chunk triggers instead of serializing in front of them.  c1b is
            # deferred below so its 0.46 MB doesn't delay chunk2's landing.
            nc.scalar.dma_start(c1a_sb[:], c1_in[:, :C1_SPLIT * NJ])
            ci = 0
            goff = 0
            for g in range(NGRP):
                grp = GRPS[g]
                ps = pspool.tile([NJ, grp * Z], f32, tag="ps")
                j0 = 0
                for kc, jchunk in enumerate(CHUNKS):
                    v4 = vpool.tile([P, max(GRPS) * max(CHUNKS) * Z], VOL_DT,
                                    padded_shape=[P, max(GRPS) * max(CHUNKS) * Z])
                    src = (vol_in[goff:goff + grp, :,
                                  j0 * Z:(j0 + jchunk) * Z]
                           .rearrange("b p f -> p b f"))
                    dst = (v4[:, :grp * jchunk * Z]
                           .rearrange("p (b f) -> p b f", b=grp))
                    nc.sync.dma_start(dst, src)
                    if ci == 1:
                        # second basis half transfers behind chunk2; its
                        # j>=18 columns aren't needed until well after
                        nc.scalar.dma_start(c1b_sb[:], c1_in[:, C1_SPLIT * NJ:])
                    ci += 1
                    v4r = v4[:, :grp * jchunk * Z].rearrange(
                        "p (b j z) -> p b j z", b=grp, j=jchunk)
                    for jj in range(jchunk):
                        j = j0 + jj
                        nc.tensor.matmul(
                            ps[:],
                            c1_col(j),
                            v4r[:, :, jj, :],
                            start=(j == 0),
                            stop=(j == Q - 1 - TAIL),
                        )
                    j0 += jchunk
                # per-bc tail: each bc's last matmuls (N=96) trail its own
                # small DMA instead of a whole 4-bc chunk, minimizing the
                # compute left after the final DMA byte lands
                for b in range(grp if TAIL else 0):
                    vt = vtail.tile([P, TAIL * Z], VOL_DT, tag="vt")
                    nc.sync.dma_start(
                        vt[:], vol_in[goff + b, :, j0 * Z:(j0 + TAIL) * Z])
                    vtr = vt[:].rearrange("p (j z) -> p j z", j=TAIL)
                    for jj in range(TAIL):
                        j = j0 + jj
                        nc.tensor.matmul(
                            ps[:, b * Z:(b + 1) * Z],
                            c1_col(j),
                            vtr[:, jj, :],
                            start=False, stop=False,
                            skip_group_check=True,
                        )
                # single drain per group: one DVE-read of the PSUM bank after
                # all its matmuls (interleaved per-bc copies serialized
                # against the next bc's matmuls via bank-overlap tracking),
                # and one out-DMA trigger instead of four 809ns ones
                ob = obuf.tile([NJ, grp * Z], OUT_DT, tag="ob")
                nc.vector.tensor_copy(ob[:], ps[:])
                half = (grp // 2) * Z
                if g == NGRP - 1 and OUT_SPLIT and half:
                    # terminal store split across two queues: the two
                    # ~0.8us descriptor-gen triggers run concurrently
                    nc.scalar.dma_start(out[:, goff * Z:goff * Z + half],
                                        ob[:, :half])
                    nc.sync.dma_start(out[:, goff * Z + half:(goff + grp) * Z],
                                      ob[:, half:])
                else:
                    nc.scalar.dma_start(out[:, goff * Z:(goff + grp) * Z],
                                        ob[:])
                goff += grp

    nc.compile()
    return nc


_NC_CACHE = None


def _get_nc():
    global _NC_CACHE
    if _NC_CACHE is None:
        _NC_CACHE = _build_nc()
    return _NC_CACHE


def kernel(cylindrical_volume):
    global LAST_RESULTS
    vol = np.asarray(cylindrical_volume)
    assert vol.shape == (B, C, R, T, Z), vol.shape
    c1_perm, ax_cat = _make_basis()
    vol_dev = _to_e3m4(vol).reshape(BC, P, Q * Z)

    nc = _get_nc()
    in_maps = [
        {"vol": vol_dev[i * BC_PER_CORE:(i + 1) * BC_PER_CORE], "c1": c1_perm}
        for i in range(N_CORES)
    ]
    import os
    try:
        res = run_bass_kernel_spmd(nc, in_maps, list(range(N_CORES)),
                                   trace=TRACE)
    except ModuleNotFoundError:
        # BASS_TRACE set but this image lacks the axon NTFF hook module;
        # rerun without tracing rather than failing
        os.environ["BASS_NEVER_TRACE"] = "1"
        try:
            res = run_bass_kernel_spmd(nc, in_maps, list(range(N_CORES)),
                                       trace=False)
        finally:
            os.environ.pop("BASS_NEVER_TRACE", None)
    LAST_RESULTS = res
    # per-core out [28, 8bc*96z] -> [8bc, 28, 96z]
    S = np.concatenate(
        [res.results[i]["out"].reshape(NJ, BC_PER_CORE, Z).transpose(1, 0, 2)
         for i in range(N_CORES)], axis=0).astype(np.float32)  # [64, 28, 96]
    out2 = np.einsum('bjz,zl->bjl', S, ax_cat)       # host stage 2: [64, 28, 22]
    ch = _combine(out2)
    return ch.reshape(B, C, 2 * MAX_N + 1, MAX_K, 2 * MAX_L + 1)

